# revision 23
# baseline (speedup 1.0000x reference)
"""Trainium2 Bass kernel for 16-head causal MHA (B=2, T=2048, D=1024, fp32).

Sharding: tensor-parallel over heads. Core c owns heads {2c, 2c+1}: it gets
Wq/Wk/Wv column slices [:, 128c:128c+128] and the Wo row slice
[128c:128c+128, :], computes its 2 heads' attention for both batch rows, and
produces a partial output [4096, 1024]; the host sums the 8 partials.

Per-core device program (per batch):
  - Q^T, K^T = W.T @ x^T  (x^T passed from host; weights stationary on PE)
  - V natural = x @ Wv    (x^T chunks stationary, Wv moving)
  - attention in S^T layout: S^T[k,q] = K^T.T @ Q^T per 128-key x 512-query
    block (the two heads' K=64 contractions are packed into PE rows 0-63 /
    64-127 and run concurrently); exp on ScalarE with scale=1/8 folded in;
    causal handled by skipping fully-masked blocks + multiplying diagonal
    blocks with a precomputed 0/1 staircase mask slice on VectorE.
  - ctx^T accumulation: lhsT = V block augmented with a ones column (M=65)
    so PSUM row 64 accumulates the softmax denominator for free.
  - normalize: reciprocal of denom row, broadcast across 64 partitions via a
    K=1 PE matmul with a ones [1,64] stationary, multiply on VectorE.
  - partial out = ctx^T.T @ Wo_c (single K=128 matmul per 128x512 block).
"""

import numpy as np

import bass_rust
from bass_rust import ScopedClock
import concourse.bass as bass
import concourse.mybir as mybir
import concourse.tile as tile

F32 = mybir.dt.float32
F32R = mybir.dt.float32r
B, T, D = 2, 2048, 1024
NCORES = 8
P = 128          # partitions / feature chunk
FC = D // P      # 8 feature chunks
QW = 512         # query block width (PSUM bank)
QN = T // QW     # 4 query blocks per batch
KC = T // P      # 16 key chunks per batch
NH = 2           # heads per core
DK = 64

# ---------------------------------------------------------------------------
# TileContext drain fix: the external walrus in this container allows only ONE
# sync wait per instruction, but Tile's closing drain packs one wait per active
# proc. Split it into a chain of single-wait drains (same semantics).
_PATCHED = False


def _patched_drain_and_barrier(self, tick_clock, wait_clock):
    nc = self.nc
    drain_inst = nc.sync.drain()
    wait_clock.add_sem_waits(
        drain_inst.ins, ScopedClock({None: tick_clock.global_clock})
    )
    si = drain_inst.ins.sync_info
    waits = list(si.on_wait) if si is not None else []
    if len(waits) > 1:
        si.on_wait = [waits[0]]
        drain_inst.ins.sync_info = si
        for w in waits[1:]:
            d2 = nc.sync.drain()
            si2 = d2.ins.sync_info
            if si2 is None:
                si2 = bass_rust.SyncInfo(on_wait=[w], on_update=[])
            else:
                si2.on_wait = [w]
            d2.ins.sync_info = si2
    nc.all_engine_barrier()
    assert self.sems is not None
    popped = nc._tile_sem_poison_stack.pop()
    assert popped is self._sem_poison
    nc.clear_and_free_semaphores(list(self.sems.allocated().values()))
    nc.all_engine_barrier()


def _apply_tile_patch():
    global _PATCHED
    if not _PATCHED:
        tile.TileContext._drain_and_barrier = _patched_drain_and_barrier
        _PATCHED = True


def _split_multi_waits(nc):
    """Post-pass: the external walrus accepts only 1 sync wait per
    instruction (2 for EventSemaphore). Tile emits more. Hoist extra waits
    onto same-engine no-ops inserted just before. For compute engines this
    is identical semantics (the engine blocks either way). For DMA triggers
    it turns queue-side waits into SP-side blocking, which is safe in this
    forward-dataflow single-block program (every wait's producer precedes
    the trigger in the scheduled stream); CoreSim re-validates no-deadlock."""
    for f in nc.m.functions:
        for bb in f.blocks:
            new = []
            for ins in bb.instructions:
                si = ins.sync_info
                if si is not None:
                    cap = 2 if isinstance(ins, mybir.InstEventSemaphore) else 1
                    waits = list(si.on_wait)
                    if len(waits) > cap:
                        for w in waits[:-cap]:
                            nop = mybir.InstNoOp(
                                name=nc.get_next_instruction_name(),
                                engine=ins.engine,
                                sync_info=bass_rust.SyncInfo(
                                    on_wait=[w], on_update=[]
                                ),
                                bass_nofuse=True,
                            )
                            nc.register_instruction(nop, overwrite=True)
                            new.append(nop)
                        si.on_wait = waits[-cap:]
                        ins.sync_info = si
                new.append(ins)
            bb.instructions = new


# ---------------------------------------------------------------------------
_PROGRAM = None


def build_program():
    global _PROGRAM
    if _PROGRAM is not None:
        return _PROGRAM
    _apply_tile_patch()
    Exp = mybir.ActivationFunctionType.Exp
    Log = mybir.ActivationFunctionType.Ln
    Copy = mybir.ActivationFunctionType.Copy

    # float32r tiles everywhere that feeds the PE: same 4-byte storage as
    # fp32, but matmuls stream at 1 cycle/row (vs 4 for fp32) when the
    # moving free dim is >= 256, at ~tf32 precision.
    nc = bass.Bass()
    xt_d = nc.declare_dram_parameter("xt", [D, B * T], F32R, isOutput=False)
    wq_d = nc.declare_dram_parameter("wq", [D, P], F32R, isOutput=False)
    wk_d = nc.declare_dram_parameter("wk", [D, P], F32R, isOutput=False)
    wv_d = nc.declare_dram_parameter("wv", [D, P], F32R, isOutput=False)
    wo_d = nc.declare_dram_parameter("wo", [P, D], F32R, isOutput=False)
    mask_d = nc.declare_dram_parameter("mask", [P, 896], F32R, isOutput=False)
    id_d = nc.declare_dram_parameter("ident", [P, P], F32R, isOutput=False)
    out_d = nc.declare_dram_parameter("out", [B * T, D], F32, isOutput=True)

    with tile.TileContext(nc) as tc:
        from contextlib import ExitStack

        ctx = ExitStack()
        with ctx:
            consts = ctx.enter_context(tc.tile_pool(name="consts", bufs=1))
            xt_pool = ctx.enter_context(tc.tile_pool(name="xt", bufs=8))
            qk_pool = ctx.enter_context(tc.tile_pool(name="qk", bufs=2))
            v_pool = ctx.enter_context(tc.tile_pool(name="v", bufs=1))
            exp_pool = ctx.enter_context(tc.tile_pool(name="exp", bufs=6))
            ctxt_pool = ctx.enter_context(tc.tile_pool(name="ctxt", bufs=2))
            ob_pool = ctx.enter_context(tc.tile_pool(name="ob", bufs=3))
            bc_pool = ctx.enter_context(tc.tile_pool(name="bc", bufs=2))
            rec_pool = ctx.enter_context(tc.tile_pool(name="rec", bufs=2))

            ps_proj = ctx.enter_context(
                tc.tile_pool(name="ps_proj", bufs=2, space="PSUM")
            )
            ps_s = ctx.enter_context(tc.tile_pool(name="ps_s", bufs=3, space="PSUM"))
            ps_ctx = ctx.enter_context(
                tc.tile_pool(name="ps_ctx", bufs=1, space="PSUM")
            )
            ps_bc = ctx.enter_context(tc.tile_pool(name="ps_bc", bufs=1, space="PSUM"))

            # ---- constants ----
            wq_sb = consts.tile([P, FC, P], F32R, tag="wq")
            wk_sb = consts.tile([P, FC, P], F32R, tag="wk")
            wv_sb = consts.tile([P, FC, P], F32R, tag="wv")
            wo_sb = consts.tile([P, D], F32R, tag="wo")
            mask_sb = consts.tile([P, 896], F32R, tag="mask")
            ident_sb = consts.tile([P, P], F32R, tag="ident")
            ones_sb = consts.tile([1, DK], F32, tag="ones")
            nc.sync.dma_start(out=wq_sb, in_=wq_d.rearrange("(f p) c -> p f c", p=P))
            nc.sync.dma_start(out=wk_sb, in_=wk_d.rearrange("(f p) c -> p f c", p=P))
            nc.sync.dma_start(out=wv_sb, in_=wv_d.rearrange("(f p) c -> p f c", p=P))
            nc.sync.dma_start(out=wo_sb, in_=wo_d[:, :])
            nc.sync.dma_start(out=mask_sb, in_=mask_d[:, :])
            nc.sync.dma_start(out=ident_sb, in_=id_d[:, :])
            nc.vector.memset(ones_sb, 1.0)

            for b in range(B):
                # ---- load x^T chunks for this batch ----
                xts = []
                for fc in range(FC):
                    xt_t = xt_pool.tile([P, T], F32R, tag="xt")
                    nc.sync.dma_start(
                        out=xt_t,
                        in_=xt_d[fc * P : (fc + 1) * P, b * T : (b + 1) * T],
                    )
                    xts.append(xt_t)

                # ---- Q^T / K^T projections ----
                qt = qk_pool.tile([P, T], F32R, tag="qt")
                kt = qk_pool.tile([P, T], F32R, tag="kt")
                vt = qk_pool.tile([P, T], F32R, tag="vt")
                for w_sb, dst in ((wq_sb, qt), (wk_sb, kt), (wv_sb, vt)):
                    for rc in range(T // QW):
                        ps = ps_proj.tile([P, QW], F32, tag="proj")
                        for fc in range(FC):
                            nc.tensor.matmul(
                                ps,
                                lhsT=w_sb[:, fc, :],
                                rhs=xts[fc][:, rc * QW : (rc + 1) * QW],
                                start=(fc == 0),
                                stop=(fc == FC - 1),
                            )
                        nc.vector.tensor_copy(dst[:, rc * QW : (rc + 1) * QW], ps)

                # ---- V natural via PE transpose of V^T (with ones cols) ----
                v_sb = v_pool.tile([P, KC, 130], F32R, tag="v")
                for kc in range(KC):
                    ps = ps_proj.tile([P, P], F32R, tag="proj", name=f"vtr{kc}")
                    nc.tensor.transpose(ps, vt[:, kc * P : (kc + 1) * P], ident_sb)
                    nc.vector.tensor_copy(v_sb[:, kc, 0:DK], ps[:, 0:DK])
                    nc.vector.tensor_copy(v_sb[:, kc, 65 : 65 + DK], ps[:, DK:P])
                    # ones columns for the denominator rows (mask col 895 == 1)
                    nc.vector.tensor_copy(v_sb[:, kc, 64:65], mask_sb[:, 895:896])
                    nc.vector.tensor_copy(v_sb[:, kc, 129:130], mask_sb[:, 895:896])

                # ---- attention ----
                ctxt = ctxt_pool.tile([P, T], F32R, tag="ctxt")
                for qn in range(QN):
                    nkc = 4 * (qn + 1)  # live key chunks (causal)
                    ctx_ps = [
                        ps_ctx.tile([65, QW], F32, tag=f"ctx{h}", name=f"ctx{h}")
                        for h in range(NH)
                    ]
                    for kc in range(nkc):
                        es = []
                        for h in range(NH):
                            s_ps = ps_s.tile([P, QW], F32, tag="s")
                            nc.tensor.matmul(
                                s_ps,
                                lhsT=kt[h * DK : (h + 1) * DK, kc * P : (kc + 1) * P],
                                rhs=qt[h * DK : (h + 1) * DK, qn * QW : (qn + 1) * QW],
                                start=True,
                                stop=True,
                            )
                            e = exp_pool.tile([P, QW], F32R, tag="exp")
                            nc.scalar.activation(out=e, in_=s_ps, func=Exp, scale=0.125)
                            j = kc - 4 * qn
                            if j >= 0:  # diagonal block: apply causal staircase
                                nc.vector.tensor_mul(
                                    e, e, mask_sb[:, 384 - 128 * j : 896 - 128 * j]
                                )
                            es.append(e)
                        for h in range(NH):
                            nc.tensor.matmul(
                                ctx_ps[h],
                                lhsT=v_sb[:, kc, h * 65 : h * 65 + 65],
                                rhs=es[h],
                                start=(kc == 0),
                                stop=(kc == nkc - 1),
                            )
                    for h in range(NH):
                        # 1/denom = exp(-ln(denom)) on ScalarE (DVE's
                        # iterative-divide reciprocal costs ~3.3us/call).
                        lnd = rec_pool.tile([1, QW], F32, tag="rec")
                        nc.scalar.activation(out=lnd, in_=ctx_ps[h][64:65, :], func=Log)
                        rcp = rec_pool.tile([1, QW], F32, tag="rcp")
                        nc.scalar.activation(out=rcp, in_=lnd, func=Exp, scale=-1.0)
                        # broadcast across 64 partitions via K=1 fp32 matmul
                        # with a ones stationary (exact).
                        bc_ps = ps_bc.tile([DK, QW], F32, tag="bc")
                        nc.tensor.matmul(
                            bc_ps, lhsT=ones_sb, rhs=rcp, start=True, stop=True
                        )
                        bc_sb = bc_pool.tile([DK, QW], F32, tag="bcs")
                        nc.scalar.activation(out=bc_sb, in_=bc_ps, func=Copy)
                        nc.vector.tensor_mul(
                            ctxt[h * DK : (h + 1) * DK, qn * QW : (qn + 1) * QW],
                            ctx_ps[h][0:DK, :],
                            bc_sb,
                        )

                # ---- output projection (partial over this core's heads) ----
                for rc in range(T // P):
                    for c2 in range(D // QW):
                        ps = ps_proj.tile([P, QW], F32, tag="proj")
                        nc.tensor.matmul(
                            ps,
                            lhsT=ctxt[:, rc * P : (rc + 1) * P],
                            rhs=wo_sb[:, c2 * QW : (c2 + 1) * QW],
                            start=True,
                            stop=True,
                        )
                        ob = ob_pool.tile([P, QW], F32, tag="ob")
                        nc.vector.tensor_copy(ob, ps)
                        nc.sync.dma_start(
                            out=out_d[
                                b * T + rc * P : b * T + (rc + 1) * P,
                                c2 * QW : (c2 + 1) * QW,
                            ],
                            in_=ob,
                        )

    _split_multi_waits(nc)
    _PROGRAM = nc
    return nc


def _make_mask():
    # mask[i, u] = 1.0 if u >= i + 384 else 0.0   (shape [128, 896])
    i = np.arange(P)[:, None]
    u = np.arange(896)[None, :]
    return (u >= i + 384).astype(np.float32)


def kernel(x, Wq, Wk, Wv, Wo):
    from concourse.bass_utils import run_bass_kernel_spmd

    x = np.asarray(x, dtype=np.float32)
    Wq = np.asarray(Wq, dtype=np.float32)
    Wk = np.asarray(Wk, dtype=np.float32)
    Wv = np.asarray(Wv, dtype=np.float32)
    Wo = np.asarray(Wo, dtype=np.float32)

    nc = build_program()
    xt = np.ascontiguousarray(x.reshape(B * T, D).T)  # [1024, 4096]
    mask = _make_mask()
    in_maps = []
    for c in range(NCORES):
        cols = slice(c * P, (c + 1) * P)
        in_maps.append(
            {
                "xt": xt,
                "wq": np.ascontiguousarray(Wq[:, cols]),
                "wk": np.ascontiguousarray(Wk[:, cols]),
                "wv": np.ascontiguousarray(Wv[:, cols]),
                "wo": np.ascontiguousarray(Wo[cols, :]),
                "mask": mask,
                "ident": np.eye(P, dtype=np.float32),
            }
        )
    res = run_bass_kernel_spmd(nc, in_maps, core_ids=list(range(NCORES)))
    acc = np.zeros((B * T, D), dtype=np.float64)
    for c in range(NCORES):
        acc += res.results[c]["out"]
    return acc.astype(np.float32).reshape(B, T, D)


if __name__ == "__main__":
    rng = np.random.default_rng(0)
    s = 1.0 / np.sqrt(D)
    ins = {
        "x": rng.standard_normal((B, T, D)).astype(np.float32),
        "Wq": (rng.standard_normal((D, D)) * s).astype(np.float32),
        "Wk": (rng.standard_normal((D, D)) * s).astype(np.float32),
        "Wv": (rng.standard_normal((D, D)) * s).astype(np.float32),
        "Wo": (rng.standard_normal((D, D)) * (1.0 / np.sqrt(D))).astype(np.float32),
    }
    out = kernel(**ins)
    print("out", out.shape, out.dtype, float(np.abs(out).max()))
